# revision 4
# baseline (speedup 1.0000x reference)
"""Trainium2 Bass kernel for nn_ClassConfusionLoss.

Self-contained: takes FULL inputs pred (64,64,128,128) f32, gt (64,64,128,128) i32,
shards the spatial W axis across 8 NeuronCores, computes per-core partial weighted
covariance M (64x64), reduces on host and applies the final row-normalization +
trace (O(C^2), negligible).

Math: the reference's global scalars num_pos and S = sum(n*w_raw) scale cov by
alpha = num_pos/S, which cancels in cov / cov.sum(axis=1). So only
M[c,k] = sum_p n_p*w_raw_p*x_pc*x_pk is needed, where x[p,c] = pred[p,c]/D_p,
D_p = sum_c pred, n_p = sum_c(gt==1), w_raw = 1+exp(E), E = sum_c x ln x
= T/D - ln D with T = sum_c pred*ln(pred).

Pixel-major layout per core (w-slab of 16 = 4 w-quad tiles):
  tile [128p=(q,b), free=(c 64, j 2, h 128)] bf16, pixel w = 4t+2q+j.
  pred: 2 cast DMAs/tile with 512B descriptors (w-pair x h contiguous).
  n: 8 accumulate-DMAs/tile into n16[p,16,256] (same-address descriptors 16
  apart -> one DMA engine each -> race-free), folded 16->1 on DVE.
  D/T: packed bf16/fp16 add-trees on DVE; Ln/Exp/Sqrt on ACT.
  z = pred * sqrt(n*w_raw/D^2) in place; G += z_jh^T @ z_jh per h-slice
  (1024 accumulating 64x64 matmuls into one PSUM bank).
Host: M = sum_cores(G); cov = M/M.sum(0-axis semantics of ref); loss.
"""

import numpy as np

B, C, W, H = 64, 64, 128, 128
NCORES = 8
WS = W // NCORES          # 16 w's per core
NT = WS // 4              # 4 w-quad tiles per core
EPS = 1e-12

_CACHE = {}


def _build_nc():
    from contextlib import ExitStack

    import concourse.bass as bass
    import concourse.tile as tile
    from concourse import bacc, mybir

    F32 = mybir.dt.float32
    BF16 = mybir.dt.bfloat16
    FP16 = mybir.dt.float16
    I32 = mybir.dt.int32
    AF = mybir.ActivationFunctionType
    OP = mybir.AluOpType

    nc = bacc.Bacc("TRN2", target_bir_lowering=False, debug=False)

    pred_t = nc.dram_tensor("pred", [B, C, WS, H], F32, kind="ExternalInput")
    gt_t = nc.dram_tensor("gt", [B, C, WS, H], I32, kind="ExternalInput")
    mout_t = nc.dram_tensor("m_out", [64, 64], F32, kind="ExternalOutput")

    # DRAM strides (elements) of the shard tensor (B, C, WS, H)
    SB_, SC_, SW_ = C * WS * H, WS * H, H

    with tile.TileContext(nc) as tc, ExitStack() as ctx:
        singles = ctx.enter_context(tc.tile_pool(name="singles", bufs=1))
        pn_pool = ctx.enter_context(tc.tile_pool(name="pn", bufs=2))
        l_pool = ctx.enter_context(tc.tile_pool(name="l", bufs=2))
        d_pool = ctx.enter_context(tc.tile_pool(name="d", bufs=2))
        n_pool = ctx.enter_context(tc.tile_pool(name="n16", bufs=2))
        sm_pool = ctx.enter_context(tc.tile_pool(name="sm", bufs=2))
        ps_g = ctx.enter_context(tc.tile_pool(name="ps_g", bufs=1, space="PSUM"))

        eps_t = singles.tile([128, 1], F32)
        nc.vector.memset(eps_t[:], EPS)

        g_ps = ps_g.tile([64, 64], F32)

        for t in range(NT):
            # ---- pred load: [128p=(q,b), (c,j,h)] bf16, 512B descriptors ----
            pn = pn_pool.tile([128, 64, 256], BF16)
            for q in range(2):
                in_ap = bass.AP(tensor=pred_t.ap().tensor,
                                offset=(4 * t + 2 * q) * SW_,
                                ap=[[SB_, 64], [SC_, 64], [1, 256]])
                nc.gpsimd.dma_start(out=pn[64 * q:64 * (q + 1)], in_=in_ap)

            # ---- n: 16-way partial accumulate DMAs (i32 -> bf16 cast) ----
            n16 = n_pool.tile([128, 16, 256], BF16)
            for q in range(2):
                for ch in range(4):
                    in_ap = bass.AP(tensor=gt_t.ap().tensor,
                                    offset=(4 * t + 2 * q) * SW_ + 16 * ch * SC_,
                                    ap=[[SB_, 64], [SC_, 16], [1, 256]])
                    out_ap = bass.AP(tensor=n16.tensor,
                                     offset=n16.offset + 64 * q * n16.ap[0][0],
                                     ap=[[n16.ap[0][0], 64], [256, 16], [1, 256]])
                    nc.gpsimd.dma_start(
                        out=out_ap, in_=in_ap,
                        accum_op=(OP.bypass if ch == 0 else OP.add))

            # ---- N-tree: 16 -> 1 on Pool (bf16, exact for n<=64) ----
            n_bf = sm_pool.tile([128, 256], BF16, tag="n")
            nc.gpsimd.tensor_tensor(out=n16[:, 0:8, :], in0=n16[:, 0:8, :],
                                    in1=n16[:, 8:16, :], op=OP.add)
            nc.gpsimd.tensor_tensor(out=n16[:, 0:4, :], in0=n16[:, 0:4, :],
                                    in1=n16[:, 4:8, :], op=OP.add)
            nc.gpsimd.tensor_tensor(out=n16[:, 0:2, :], in0=n16[:, 0:2, :],
                                    in1=n16[:, 2:4, :], op=OP.add)
            nc.gpsimd.tensor_tensor(out=n_bf[:], in0=n16[:, 0, :],
                                    in1=n16[:, 1, :], op=OP.add)

            # ---- D-tree: sum_c pred (fp16 scratch, f32 final) ----
            dscr = d_pool.tile([128, 32, 256], FP16)
            d_f = sm_pool.tile([128, 256], F32, tag="d")
            nc.vector.tensor_tensor(out=dscr[:], in0=pn[:, 0:32, :],
                                    in1=pn[:, 32:64, :], op=OP.add)
            nc.vector.tensor_tensor(out=dscr[:, 0:16, :], in0=dscr[:, 0:16, :],
                                    in1=dscr[:, 16:32, :], op=OP.add)
            nc.vector.tensor_tensor(out=dscr[:, 0:8, :], in0=dscr[:, 0:8, :],
                                    in1=dscr[:, 8:16, :], op=OP.add)
            nc.vector.tensor_tensor(out=dscr[:, 0:4, :], in0=dscr[:, 0:4, :],
                                    in1=dscr[:, 4:8, :], op=OP.add)
            nc.vector.tensor_tensor(out=dscr[:, 0:2, :], in0=dscr[:, 0:2, :],
                                    in1=dscr[:, 2:4, :], op=OP.add)
            nc.vector.tensor_tensor(out=d_f[:], in0=dscr[:, 0, :],
                                    in1=dscr[:, 1, :], op=OP.add)

            # ---- L = ln(pred + eps); pl = pred * L (in place); T-tree ----
            L = l_pool.tile([128, 64, 256], FP16)
            nc.scalar.activation(L[:], pn[:], AF.Ln, bias=eps_t[:], scale=1.0)
            nc.vector.tensor_mul(L[:], pn[:], L[:])
            t_f = sm_pool.tile([128, 256], F32, tag="t")
            nc.vector.tensor_tensor(out=L[:, 0:32, :], in0=L[:, 0:32, :],
                                    in1=L[:, 32:64, :], op=OP.add)
            nc.vector.tensor_tensor(out=L[:, 0:16, :], in0=L[:, 0:16, :],
                                    in1=L[:, 16:32, :], op=OP.add)
            nc.vector.tensor_tensor(out=L[:, 0:8, :], in0=L[:, 0:8, :],
                                    in1=L[:, 8:16, :], op=OP.add)
            nc.vector.tensor_tensor(out=L[:, 0:4, :], in0=L[:, 0:4, :],
                                    in1=L[:, 4:8, :], op=OP.add)
            nc.vector.tensor_tensor(out=L[:, 0:2, :], in0=L[:, 0:2, :],
                                    in1=L[:, 2:4, :], op=OP.add)
            nc.vector.tensor_tensor(out=t_f[:], in0=L[:, 0, :],
                                    in1=L[:, 1, :], op=OP.add)

            # ---- per-pixel weight rs = sqrt(n*(1+exp(T/D-lnD))/D^2), sqrt-free:
            # rs = exp(0.5*ln(u) - lnD), u = n*(1+exp(E)). All ACT funcs are
            # ln/exp (one act table set -> no reloads).
            dr = sm_pool.tile([128, 256], F32, tag="dr")
            nc.vector.reciprocal(dr[:], d_f[:])
            lnd = sm_pool.tile([128, 256], F32, tag="lnd")
            nc.scalar.activation(lnd[:], d_f[:], AF.Ln, bias=eps_t[:], scale=1.0)
            e_f = sm_pool.tile([128, 256], F32, tag="e")
            nc.vector.tensor_mul(e_f[:], t_f[:], dr[:])
            nc.vector.tensor_tensor(out=e_f[:], in0=e_f[:], in1=lnd[:],
                                    op=OP.subtract)
            ee = sm_pool.tile([128, 256], F32, tag="ee")
            nc.scalar.activation(ee[:], e_f[:], AF.Exp, bias=0.0, scale=1.0)
            u_f = sm_pool.tile([128, 256], F32, tag="u")
            nc.vector.scalar_tensor_tensor(out=u_f[:], in0=ee[:], scalar=1.0,
                                           in1=n_bf[:], op0=OP.add, op1=OP.mult)
            lnu = sm_pool.tile([128, 256], F32, tag="lnu")
            nc.scalar.activation(lnu[:], u_f[:], AF.Ln, bias=eps_t[:], scale=1.0)
            lu2 = sm_pool.tile([128, 256], F32, tag="lu2")
            nc.vector.scalar_tensor_tensor(out=lu2[:], in0=lnd[:], scalar=-2.0,
                                           in1=lnu[:], op0=OP.mult, op1=OP.add)
            rs = sm_pool.tile([128, 256], FP16, tag="rs")
            nc.scalar.activation(rs[:], lu2[:], AF.Exp, bias=0.0, scale=0.5)

            # ---- z = pred * rs (in place, rs broadcast over c; jh-split) ----
            for half in range(2):
                sl = slice(128 * half, 128 * (half + 1))
                rs_b = bass.AP(tensor=rs.tensor, offset=rs.offset + 128 * half,
                               ap=[rs.ap[0], [0, 64], [1, 128]])
                nc.vector.tensor_mul(pn[:, :, sl], pn[:, :, sl], rs_b)

            # ---- G += z_jh^T @ z_jh per (j,h) slice ----
            for jh in range(256):
                z_ap = bass.AP(tensor=pn.tensor, offset=pn.offset + jh,
                               ap=[pn.ap[0], [256, 64]])
                nc.tensor.matmul(g_ps[:], z_ap, z_ap,
                                 start=(t == 0 and jh == 0),
                                 stop=(t == NT - 1 and jh == 255),
                                 skip_group_check=True)

        g_sb = singles.tile([64, 64], F32)
        nc.vector.tensor_copy(g_sb[:], g_ps[:])
        nc.sync.dma_start(out=mout_t.ap(), in_=g_sb[:])

    nc.compile()
    return nc


def _get_nc():
    if "nc" not in _CACHE:
        _CACHE["nc"] = _build_nc()
    return _CACHE["nc"]


def kernel(pred: np.ndarray, gt: np.ndarray) -> np.ndarray:
    from concourse.bass_utils import run_bass_kernel_spmd

    pred = np.ascontiguousarray(pred, dtype=np.float32)
    gt = np.ascontiguousarray(gt, dtype=np.int32)
    nc = _get_nc()

    in_maps = []
    for s in range(NCORES):
        in_maps.append({
            "pred": np.ascontiguousarray(pred[:, :, s * WS:(s + 1) * WS, :]),
            "gt": np.ascontiguousarray(gt[:, :, s * WS:(s + 1) * WS, :]),
        })
    res = run_bass_kernel_spmd(nc, in_maps, core_ids=list(range(NCORES)))

    M = np.zeros((64, 64), dtype=np.float64)
    for r in res.results:
        M += r["m_out"].astype(np.float64)
    cov = M / M.sum(axis=1)
    return np.float32((cov.sum() - np.trace(cov)) / C)
